# revision 11
# baseline (speedup 1.0000x reference)
"""Trainium2 Bass kernel for nn_DomainDiscriminator.

Network: conv(512->256,k3,s3,p1) -> BN -> conv(256->128,k3,s3,p1) -> BN
         -> reshape -> 12-layer MLP (3200->...->1, no nonlinearities) -> sigmoid.
Input x: [64, 512, 40, 40] f32.  Output: [64, 1] f32.

Strategy (8 NeuronCores):
 - Data-parallel batch shard (8 per core) for the convs.
 - stride==kernel==3 convs are non-overlapping patch matmuls. Conv1 patches are
   built host-side (space-to-depth, free); conv2 patches are read straight out
   of SBUF with strided access patterns (boundary-split matmuls, no im2col).
 - Training-mode BN: conv bias is absorbed exactly by BN; BN1 stats via a 2KB
   AllReduce; BN2 stats computed redundantly after an AllGather of the conv2
   raw output.
 - The 12 linear layers have no activations between them, so they compose on
   the host (fp64) into a single [3200] vector + scalar bias; the device does
   one 25-chunk matvec + sigmoid.
 - Convs run in bf16 (BN re-normalizes, keeping error ~2e-3); the final matvec
   in float32r.
"""

import os
import sys

sys.path.insert(0, "/opt/trn_rl_repo")

import numpy as np

import concourse.bass as bass
import concourse.mybir as mybir
import concourse.tile as tile
from concourse import bacc
from concourse.bass_utils import run_bass_kernel_spmd

F32 = mybir.dt.float32
F32R = mybir.dt.float32r
BF16 = mybir.dt.bfloat16

NCORES = 8
BL = 8              # batch per core
B = 64              # full batch
EPS = 1e-5

# conv1: [BL,512,40,40] -> [BL,256,14,14]; conv2: -> [BL,128,5,5]
P1 = 196            # 14*14 positions
P2 = 25             # 5*5 positions
NPT = 4             # conv1 psum tiles (2 batch each)
PTW = 2 * P1        # 392 columns per conv1 psum tile

_CACHE = {}


# ----------------------------------------------------------------------------
# device program
# ----------------------------------------------------------------------------

def _build():
    nc = bacc.Bacc("TRN2", target_bir_lowering=False, debug=False,
                   enable_asserts=True, num_devices=NCORES)

    xprep = nc.dram_tensor("xprep", [4, NPT, 128, 9 * PTW], BF16,
                           kind="ExternalInput")
    w1p = nc.dram_tensor("w1p", [128, 36, 256], BF16, kind="ExternalInput")
    w2p = nc.dram_tensor("w2p", [128, 18, 128], BF16, kind="ExternalInput")
    weffp = nc.dram_tensor("weffp", [128, 25], F32, kind="ExternalInput")
    bprep = nc.dram_tensor("bprep", [128, 7], F32, kind="ExternalInput")
    out = nc.dram_tensor("out", [B, 1], F32, kind="ExternalOutput")
    debug = bool(int(os.environ.get("KERNEL_DEBUG", "0")))
    if debug:
        dbg_h1 = nc.dram_tensor("dbg_h1", [2, 128, 1568], F32, kind="ExternalOutput")
        dbg_g2 = nc.dram_tensor("dbg_g2", [128, 1600], F32, kind="ExternalOutput")
        dbg_st1 = nc.dram_tensor("dbg_st1", [128, 4], F32, kind="ExternalOutput")

    # bprep columns: bn1_g (2), bn1_b (2), bn2_g, bn2_b, beff(row 0)
    BC_BN1G, BC_BN1B, BC_BN2G, BC_BN2B, BC_BEFF = 0, 2, 4, 5, 6

    with tile.TileContext(nc) as tc:
        with tc.tile_pool(name="wp", bufs=1) as wp, \
             tc.tile_pool(name="xp", bufs=4) as xp, \
             tc.tile_pool(name="hp", bufs=1) as hp, \
             tc.tile_pool(name="sp", bufs=1) as sp, \
             tc.tile_pool(name="cps", bufs=4, space="PSUM") as cps, \
             tc.tile_pool(name="c2p", bufs=1, space="PSUM") as c2p, \
             tc.tile_pool(name="zp", bufs=1, space="PSUM") as zp, \
             tc.tile_pool(name="dram", bufs=1, space="DRAM") as dram:

            # ---------------- weight/bias loads -------------------------
            w1sb = wp.tile([128, 36 * 256], BF16)
            w1r = w1p.ap().rearrange("p a b -> p (a b)")
            nc.sync.dma_start(w1sb[:, 0:9 * 256], w1r[:, 0:9 * 256])

            # ---------------- conv1 -------------------------------------
            h1sb = [hp.tile([128, 4 * PTW], BF16, name=f"h1_{mt}") for mt in range(2)]
            for pt in range(NPT):
                ps = [cps.tile([128, PTW], F32, name="c1ps", tag="c1ps")
                      for _ in range(2)]
                for cb in range(4):
                    xt = xp.tile([128, 9 * PTW], BF16, name="xt", tag="xt")
                    nc.sync.dma_start(xt[:], xprep.ap()[cb, pt])
                    if pt == 0 and cb < 3:
                        # stream the rest of w1 behind the first x chunk
                        sl = slice((cb + 1) * 9 * 256, (cb + 2) * 9 * 256)
                        nc.sync.dma_start(w1sb[:, sl], w1r[:, sl])
                    xtr = xt[:].rearrange("p (k c) -> p k c", k=9)
                    for kij in range(9):
                        rhs = xtr[:, kij]
                        for mt in range(2):
                            lhsT = w1sb[:, (cb * 9 + kij) * 256 + mt * 128:
                                        (cb * 9 + kij) * 256 + (mt + 1) * 128]
                            nc.tensor.matmul(ps[mt][:], lhsT, rhs,
                                             start=(cb == 0 and kij == 0),
                                             stop=(cb == 3 and kij == 8))
                for mt in range(2):
                    nc.vector.tensor_copy(
                        h1sb[mt][:, pt * PTW:(pt + 1) * PTW], ps[mt][:])

            # late loads (behind the x stream on the SP ring)
            w2sb = wp.tile([128, 18 * 128], BF16)
            nc.sync.dma_start(w2sb[:], w2p.ap().rearrange("p a b -> p (a b)"))
            weff = wp.tile([128, 25], F32)
            nc.sync.dma_start(weff[:], weffp.ap())
            bsb = wp.tile([128, 7], F32)
            nc.sync.dma_start(bsb[:], bprep.ap())

            # ---------------- BN1 stats + AllReduce ---------------------
            # bounce DMAs ride the Scalar HWDGE ring so they are not stuck
            # behind bulk loads on the SP ring
            scratch = sp.tile([128, 1600], F32)
            st_in = sp.tile([128, 4], F32)
            for mt in range(2):
                h = h1sb[mt][:]
                nc.vector.reduce_sum(st_in[:, 2 * mt:2 * mt + 1], h,
                                     axis=mybir.AxisListType.X)
                nc.scalar.activation(scratch[:, :4 * PTW], h,
                                     mybir.ActivationFunctionType.Square,
                                     accum_out=st_in[:, 2 * mt + 1:2 * mt + 2])
            bn1_in = dram.tile([128, 4], F32)
            bn1_out = dram.tile([NCORES, 128, 4], F32, addr_space="Shared")
            nc.scalar.dma_start(bn1_in[:], st_in[:])
            nc.gpsimd.collective_compute(
                "AllGather", mybir.AluOpType.bypass,
                replica_groups=[list(range(NCORES))],
                ins=[bn1_in.opt()], outs=[bn1_out.opt()])
            stg = sp.tile([128, NCORES * 4], F32)
            nc.scalar.dma_start(
                stg[:].rearrange("p (r t) -> p r t", r=NCORES),
                bass.AP(bn1_out.tensor, 0, [[4, 128], [128 * 4, NCORES], [1, 4]]))
            # tree-sum the 8 rank blocks: 8x4 -> 4x4 -> 2x4 -> 1x4
            stgr = stg[:].rearrange("p (r t) -> p r t", r=NCORES)
            for half in (4, 2, 1):
                nc.vector.tensor_tensor(
                    stgr[:, 0:half], stgr[:, 0:half], stgr[:, half:2 * half],
                    op=mybir.AluOpType.add)
            st1 = stg[:, 0:4]

            # ---------------- BN1 scale/shift + apply -------------------
            def bn_coeffs(pool, stats_sum, stats_sq, count, g_ap, b_ap, name):
                """returns (scale, shift) [p,1] tiles; stats_* are [p,1] APs"""
                p = stats_sum.shape[0]
                t = pool.tile([p, 6], F32, name=f"bn_{name}")
                mean, msq, vpe, sd, r, tn = (t[:, i:i + 1] for i in range(6))
                nc.vector.tensor_scalar(mean, stats_sum, 1.0 / count, None,
                                        op0=mybir.AluOpType.mult)
                nc.vector.tensor_scalar(vpe, stats_sq, 1.0 / count, None,
                                        op0=mybir.AluOpType.mult)
                nc.vector.tensor_tensor(msq, mean, mean, op=mybir.AluOpType.mult)
                nc.vector.tensor_tensor(vpe, vpe, msq, op=mybir.AluOpType.subtract)
                nc.vector.tensor_scalar(vpe, vpe, EPS, None, op0=mybir.AluOpType.add)
                nc.scalar.activation(sd, vpe, mybir.ActivationFunctionType.Sqrt)
                nc.vector.reciprocal(r, sd)
                # one Newton step: r *= 1.5 - 0.5*vpe*r*r
                nc.vector.tensor_tensor(tn, r, r, op=mybir.AluOpType.mult)
                nc.vector.tensor_tensor(tn, tn, vpe, op=mybir.AluOpType.mult)
                nc.vector.tensor_scalar(tn, tn, -0.5, 1.5,
                                        op0=mybir.AluOpType.mult,
                                        op1=mybir.AluOpType.add)
                nc.vector.tensor_tensor(r, r, tn, op=mybir.AluOpType.mult)
                co = pool.tile([p, 2], F32, name=f"bnc_{name}")
                scale, shift = co[:, 0:1], co[:, 1:2]
                nc.vector.tensor_tensor(scale, g_ap, r, op=mybir.AluOpType.mult)
                nc.vector.tensor_tensor(tn, mean, scale, op=mybir.AluOpType.mult)
                nc.vector.tensor_tensor(shift, b_ap, tn, op=mybir.AluOpType.subtract)
                return scale, shift

            for mt in range(2):
                scale, shift = bn_coeffs(
                    sp, st1[:, 2 * mt:2 * mt + 1], st1[:, 2 * mt + 1:2 * mt + 2],
                    B * P1, bsb[:, BC_BN1G + mt:BC_BN1G + mt + 1],
                    bsb[:, BC_BN1B + mt:BC_BN1B + mt + 1], f"bn1_{mt}")
                nc.vector.tensor_scalar(h1sb[mt][:], h1sb[mt][:],
                                        scale, shift,
                                        op0=mybir.AluOpType.mult,
                                        op1=mybir.AluOpType.add)

            if debug:
                for mt in range(2):
                    dh = sp.tile([128, 1568], F32, name=f"dh{mt}")
                    nc.vector.tensor_copy(dh[:], h1sb[mt][:])
                    nc.sync.dma_start(dbg_h1.ap()[mt], dh[:])
                nc.sync.dma_start(dbg_st1.ap(), st1[:])

            # ---------------- conv2 (strided APs, boundary split) -------
            # psum layout (i2, j2, n): n innermost; two parallel psum chains
            # (one per input-channel block), summed by DVE at the end
            kij_order = [(1, 1), (1, 2), (2, 1), (2, 2), (0, 1), (0, 2),
                         (1, 0), (2, 0), (0, 0)]
            c2ps = []
            for cb2 in range(2):
                cp = c2p.tile([128, P2 * BL], F32, name=f"c2ps{cb2}",
                              tag=f"c2ps{cb2}")
                c2ps.append(cp)
                c2r = cp[:].rearrange("p (i j n) -> p i j n", i=5, j=5, n=BL)
                hr = h1sb[cb2][:].rearrange(
                    "p (n i j) -> p n i j", n=BL, i=14, j=14).transpose([0, 2, 3, 1])
                for cnt, (ki, kj) in enumerate(kij_order):
                    ilo = 1 if ki == 0 else 0
                    jlo = 1 if kj == 0 else 0
                    src = hr[:, 3 * ilo + ki - 1:14:3, 3 * jlo + kj - 1:14:3, :]
                    dst = c2r[:, ilo:, jlo:, :]
                    lhsT = w2sb[:, (cb2 * 9 + ki * 3 + kj) * 128:
                                (cb2 * 9 + ki * 3 + kj + 1) * 128]
                    nc.tensor.matmul(dst, lhsT, src, start=(cnt == 0),
                                     stop=(cnt == 8), skip_group_check=True)
            # DVE has a single PSUM read port: go through SBUF for the add
            c2half = sp.tile([128, BL * P2], F32)
            nc.vector.tensor_copy(c2half[:], c2ps[0][:])
            c2sb = sp.tile([128, BL * P2], BF16)
            nc.vector.tensor_tensor(c2sb[:], c2half[:], c2ps[1][:],
                                    op=mybir.AluOpType.add)

            # ---------------- AllGather conv2 raw -----------------------
            ag_in = dram.tile([128, BL * P2], BF16)
            ag_out = dram.tile([NCORES, 128, BL * P2], BF16, addr_space="Shared")
            nc.scalar.dma_start(ag_in[:], c2sb[:])
            nc.gpsimd.collective_compute(
                "AllGather", mybir.AluOpType.bypass,
                replica_groups=[list(range(NCORES))],
                ins=[ag_in.opt()], outs=[ag_out.opt()])
            g2 = sp.tile([128, B * P2], BF16)
            nc.scalar.dma_start(
                g2[:].rearrange("p (r t) -> p r t", r=NCORES),
                bass.AP(ag_out.tensor, 0,
                        [[BL * P2, 128], [128 * BL * P2, NCORES], [1, BL * P2]]))

            # ---------------- BN2 (redundant, full batch) ---------------
            st2 = sp.tile([128, 2], F32)
            nc.vector.reduce_sum(st2[:, 0:1], g2[:], axis=mybir.AxisListType.X)
            nc.scalar.activation(scratch[:, :B * P2], g2[:],
                                 mybir.ActivationFunctionType.Square,
                                 accum_out=st2[:, 1:2])
            scale2, shift2 = bn_coeffs(
                sp, st2[:, 0:1], st2[:, 1:2], B * P2,
                bsb[:, BC_BN2G:BC_BN2G + 1], bsb[:, BC_BN2B:BC_BN2B + 1], "bn2")
            g2a = sp.tile([128, B * P2], F32R)
            nc.vector.tensor_scalar(g2a[:], g2[:], scale2, shift2,
                                    op0=mybir.AluOpType.mult,
                                    op1=mybir.AluOpType.add)
            if debug:
                dg = sp.tile([128, 1600], F32, name="dg")
                nc.vector.tensor_copy(dg[:], g2a[:].bitcast(F32))
                nc.sync.dma_start(dbg_g2.ap(), dg[:])

            # ---------------- collapsed MLP: one matvec + sigmoid -------
            # z[n] = sum_{c,ij} weff[c,ij] * g2a[c, (r,ij,n8)] ; out = sigmoid(z + beff)
            weffr = wp.tile([128, 25], F32R)
            nc.vector.tensor_copy(weffr[:], weff[:])
            g2v = g2a[:].rearrange("p (r i n) -> p r i n", r=NCORES, i=P2)
            zps = zp.tile([1, B], F32)
            for ij in range(P2):
                nc.tensor.matmul(zps[:], weffr[:, ij:ij + 1], g2v[:, :, ij, :],
                                 start=(ij == 0), stop=(ij == P2 - 1))
            osb = sp.tile([1, B], F32)
            nc.scalar.activation(osb[:], zps[:],
                                 mybir.ActivationFunctionType.Sigmoid,
                                 bias=bsb[0:1, BC_BEFF:BC_BEFF + 1])
            nc.sync.dma_start(bass.AP(out, 0, [[1, 1], [1, B]]), osb[:])

    nc.compile()
    return nc


# ----------------------------------------------------------------------------
# host-side input prep
# ----------------------------------------------------------------------------

def _prep_inputs(inputs):
    import ml_dtypes
    f = np.float32
    bf = ml_dtypes.bfloat16
    x = np.asarray(inputs["x"], dtype=f)

    # conv1 patches, per core: [4cb, 4pt, 128c, 9kij * 392]
    xpad = np.zeros((B, 512, 42, 42), dtype=bf)
    xpad[:, :, 1:41, 1:41] = x.astype(bf)
    # [n, cb, c, i, ki, j, kj] -> [cb, c, ki, kj, n, i, j]
    xv = xpad.reshape(B, 4, 128, 14, 3, 14, 3).transpose(1, 2, 4, 6, 0, 3, 5)

    w1 = np.asarray(inputs["conv1_w"], dtype=f)          # [256, 512, 3, 3]
    w1p = np.ascontiguousarray(
        w1.reshape(256, 4, 128, 9).transpose(2, 1, 3, 0)).reshape(128, 36, 256).astype(bf)
    w2 = np.asarray(inputs["conv2_w"], dtype=f)          # [128, 256, 3, 3]
    w2p = np.ascontiguousarray(
        w2.reshape(128, 2, 128, 9).transpose(2, 1, 3, 0)).reshape(128, 18, 128).astype(bf)

    # compose the 12 affine layers (no nonlinearities) into [3200] + scalar
    M = np.asarray(inputs["w14"], dtype=np.float64)      # [1, 2]
    beff = np.asarray(inputs["b14"], dtype=np.float64).copy()  # [1]
    for li in range(13, 2, -1):                          # w13 .. w3
        beff += M @ np.asarray(inputs[f"b{li}"], dtype=np.float64)
        M = M @ np.asarray(inputs[f"w{li}"], dtype=np.float64)
    weff = M.reshape(3200).astype(f)                     # order f = c*25 + ij
    weffp = np.ascontiguousarray(weff.reshape(128, 25))
    beff_f = float(beff[0])

    bn1_g = np.asarray(inputs["bn1_g"], dtype=f)
    bn1_b = np.asarray(inputs["bn1_b"], dtype=f)
    bn2_g = np.asarray(inputs["bn2_g"], dtype=f)
    bn2_b = np.asarray(inputs["bn2_b"], dtype=f)

    bp = np.zeros((128, 7), dtype=f)
    bp[:, 0:2] = bn1_g.reshape(2, 128).T
    bp[:, 2:4] = bn1_b.reshape(2, 128).T
    bp[:, 4] = bn2_g
    bp[:, 5] = bn2_b
    bp[0, 6] = beff_f

    in_maps = []
    for r in range(NCORES):
        xr = np.ascontiguousarray(
            xv[:, :, :, :, r * BL:(r + 1) * BL]        # [4,128,3,3,8,14,14]
            .reshape(4, 128, 9, NPT, PTW)
            .transpose(0, 3, 1, 2, 4)                  # [4cb, 4pt, 128, 9, 392]
        ).reshape(4, NPT, 128, 9 * PTW)
        in_maps.append({
            "xprep": xr, "w1p": w1p, "w2p": w2p,
            "weffp": weffp, "bprep": bp,
        })
    return in_maps


def kernel(**inputs):
    if "nc" not in _CACHE:
        _CACHE["nc"] = _build()
    nc = _CACHE["nc"]
    in_maps = _prep_inputs(inputs)
    trace = bool(int(os.environ.get("KERNEL_TRACE", "0")))
    if trace:
        import ntff_shim
        ntff_shim.install()
    res = run_bass_kernel_spmd(nc, in_maps, core_ids=list(range(NCORES)),
                               trace=trace)
    _CACHE["last_result"] = res
    return res.results[0]["out"]


# revision 14
# speedup vs baseline: 1.0132x; 1.0132x over previous
"""Trainium2 Bass kernel for nn_DomainDiscriminator.

Network: conv(512->256,k3,s3,p1) -> BN -> conv(256->128,k3,s3,p1) -> BN
         -> reshape -> 12-layer MLP (3200->...->1, no nonlinearities) -> sigmoid.
Input x: [64, 512, 40, 40] f32.  Output: [64, 1] f32.

Strategy (8 NeuronCores):
 - Data-parallel batch shard (8 per core) for the convs.
 - stride==kernel==3 convs are non-overlapping patch matmuls. Conv1 patches are
   built host-side (space-to-depth, free); conv2 patches are read straight out
   of SBUF with strided access patterns (boundary-split matmuls, no im2col).
 - Training-mode BN: conv bias is absorbed exactly by BN; BN1 stats via a 2KB
   AllReduce; BN2 stats computed redundantly after an AllGather of the conv2
   raw output.
 - The 12 linear layers have no activations between them, so they compose on
   the host (fp64) into a single [3200] vector + scalar bias; the device does
   one 25-chunk matvec + sigmoid.
 - Convs run in bf16 (BN re-normalizes, keeping error ~2e-3); the final matvec
   in float32r.
"""

import os
import sys

sys.path.insert(0, "/opt/trn_rl_repo")

import numpy as np

import concourse.bass as bass
import concourse.mybir as mybir
import concourse.tile as tile
from concourse import bacc
from concourse.bass_utils import run_bass_kernel_spmd

F32 = mybir.dt.float32
F32R = mybir.dt.float32r
BF16 = mybir.dt.bfloat16

NCORES = 8
BL = 8              # batch per core
B = 64              # full batch
EPS = 1e-5

# conv1: [BL,512,40,40] -> [BL,256,14,14]; conv2: -> [BL,128,5,5]
P1 = 196            # 14*14 positions
P2 = 25             # 5*5 positions
NPT = 4             # conv1 psum tiles (2 batch each)
PTW = 2 * P1        # 392 columns per conv1 psum tile

_CACHE = {}


# ----------------------------------------------------------------------------
# device program
# ----------------------------------------------------------------------------

def _build():
    nc = bacc.Bacc("TRN2", target_bir_lowering=False, debug=False,
                   enable_asserts=True, num_devices=NCORES)

    xprep = nc.dram_tensor("xprep", [4, NPT, 128, 9 * PTW], BF16,
                           kind="ExternalInput")
    w1p = nc.dram_tensor("w1p", [128, 36, 256], BF16, kind="ExternalInput")
    w2p = nc.dram_tensor("w2p", [128, 18, 128], BF16, kind="ExternalInput")
    weffp = nc.dram_tensor("weffp", [128, 26], F32, kind="ExternalInput")
    bprep = nc.dram_tensor("bprep", [128, 7], F32, kind="ExternalInput")
    out = nc.dram_tensor("out", [B, 1], F32, kind="ExternalOutput")
    debug = bool(int(os.environ.get("KERNEL_DEBUG", "0")))
    if debug:
        dbg_h1 = nc.dram_tensor("dbg_h1", [2, 128, 1568], F32, kind="ExternalOutput")
        dbg_g2 = nc.dram_tensor("dbg_g2", [128, 1600], F32, kind="ExternalOutput")
        dbg_st1 = nc.dram_tensor("dbg_st1", [128, 4], F32, kind="ExternalOutput")

    # bprep columns: bn1_g (2), bn1_b (2), bn2_g, bn2_b, beff(row 0)
    BC_BN1G, BC_BN1B, BC_BN2G, BC_BN2B, BC_BEFF = 0, 2, 4, 5, 6

    with tile.TileContext(nc) as tc:
        with tc.tile_pool(name="wp", bufs=1) as wp, \
             tc.tile_pool(name="xp", bufs=4) as xp, \
             tc.tile_pool(name="hp", bufs=1) as hp, \
             tc.tile_pool(name="sp", bufs=1) as sp, \
             tc.tile_pool(name="cps", bufs=4, space="PSUM") as cps, \
             tc.tile_pool(name="c2p", bufs=1, space="PSUM") as c2p, \
             tc.tile_pool(name="zp", bufs=1, space="PSUM") as zp, \
             tc.tile_pool(name="dram", bufs=1, space="DRAM") as dram:

            # ---------------- weight/bias loads -------------------------
            w1sb = wp.tile([128, 36 * 256], BF16)
            w1r = w1p.ap().rearrange("p a b -> p (a b)")
            nc.sync.dma_start(w1sb[:, 0:9 * 256], w1r[:, 0:9 * 256])

            # ncfw warm-up: a tiny AllGather nobody consumes; hides the
            # ~12us TOPSP cold-start under conv1
            warm_in = dram.tile([1, 4], F32)
            warm_out = dram.tile([NCORES, 1, 4], F32, addr_space="Shared")
            dummy = sp.tile([1, 4], F32)
            nc.gpsimd.memset(dummy[:], 0.0)
            nc.scalar.dma_start(warm_in[:], dummy[:])
            nc.gpsimd.collective_compute(
                "AllGather", mybir.AluOpType.bypass,
                replica_groups=[list(range(NCORES))],
                ins=[warm_in.opt()], outs=[warm_out.opt()])
            # ACT table preloads (Sqrt/Sigmoid) while ACT is idle
            nc.scalar.activation(dummy[:, 0:1], dummy[:, 1:2],
                                 mybir.ActivationFunctionType.Square)
            nc.scalar.activation(dummy[:, 0:1], dummy[:, 1:2],
                                 mybir.ActivationFunctionType.Sqrt)
            nc.scalar.activation(dummy[:, 0:1], dummy[:, 1:2],
                                 mybir.ActivationFunctionType.Sigmoid)

            # ---------------- conv1 -------------------------------------
            h1sb = [hp.tile([128, 4 * PTW], BF16, name=f"h1_{mt}") for mt in range(2)]
            for pt in range(NPT):
                ps = [cps.tile([128, PTW], F32, name="c1ps", tag="c1ps")
                      for _ in range(2)]
                for cb in range(4):
                    xt = xp.tile([128, 9 * PTW], BF16, name="xt", tag="xt")
                    nc.sync.dma_start(xt[:], xprep.ap()[cb, pt])
                    if pt == 0 and cb < 3:
                        # stream the rest of w1 behind the first x chunk
                        sl = slice((cb + 1) * 9 * 256, (cb + 2) * 9 * 256)
                        nc.sync.dma_start(w1sb[:, sl], w1r[:, sl])
                    xtr = xt[:].rearrange("p (k c) -> p k c", k=9)
                    for kij in range(9):
                        rhs = xtr[:, kij]
                        for mt in range(2):
                            lhsT = w1sb[:, (cb * 9 + kij) * 256 + mt * 128:
                                        (cb * 9 + kij) * 256 + (mt + 1) * 128]
                            nc.tensor.matmul(ps[mt][:], lhsT, rhs,
                                             start=(cb == 0 and kij == 0),
                                             stop=(cb == 3 and kij == 8))
                for mt in range(2):
                    nc.vector.tensor_copy(
                        h1sb[mt][:, pt * PTW:(pt + 1) * PTW], ps[mt][:])

            # late loads (behind the x stream on the SP ring)
            w2sb = wp.tile([128, 18 * 128], BF16)
            nc.sync.dma_start(w2sb[:], w2p.ap().rearrange("p a b -> p (a b)"))
            weff = wp.tile([128, 26], F32)
            nc.sync.dma_start(weff[:], weffp.ap())
            bsb = wp.tile([128, 7], F32)
            nc.sync.dma_start(bsb[:], bprep.ap())

            # ---------------- BN1 stats + AllReduce ---------------------
            # bounce DMAs ride the Scalar HWDGE ring so they are not stuck
            # behind bulk loads on the SP ring
            scratch = sp.tile([128, 1600], F32)
            st_in = sp.tile([128, 4], F32)
            for mt in range(2):
                h = h1sb[mt][:]
                nc.vector.reduce_sum(st_in[:, mt:mt + 1], h,
                                     axis=mybir.AxisListType.X)
                nc.scalar.activation(scratch[:, :4 * PTW], h,
                                     mybir.ActivationFunctionType.Square,
                                     accum_out=st_in[:, 2 + mt:3 + mt])
            bn1_in = dram.tile([128, 4], F32)
            bn1_out = dram.tile([NCORES, 128, 4], F32, addr_space="Shared")
            nc.scalar.dma_start(bn1_in[:], st_in[:])
            nc.gpsimd.collective_compute(
                "AllGather", mybir.AluOpType.bypass,
                replica_groups=[list(range(NCORES))],
                ins=[bn1_in.opt()], outs=[bn1_out.opt()])
            stg = sp.tile([128, NCORES * 4], F32)
            nc.scalar.dma_start(
                stg[:].rearrange("p (r t) -> p r t", r=NCORES),
                bass.AP(bn1_out.tensor, 0, [[4, 128], [128 * 4, NCORES], [1, 4]]))
            # tree-sum the 8 rank blocks: 8x4 -> 4x4 -> 2x4 -> 1x4
            stgr = stg[:].rearrange("p (r t) -> p r t", r=NCORES)
            for half in (4, 2, 1):
                nc.vector.tensor_tensor(
                    stgr[:, 0:half], stgr[:, 0:half], stgr[:, half:2 * half],
                    op=mybir.AluOpType.add)
            st1 = stg[:, 0:4]

            # ---------------- BN1 scale/shift + apply -------------------
            def bn_coeffs(pool, stats_sum, stats_sq, count, g_ap, b_ap, name):
                """returns (scale, shift) [p,w] tiles; stats_* are [p,w] APs"""
                p, w = stats_sum.shape
                t = pool.tile([p, 6 * w], F32, name=f"bn_{name}")
                mean, msq, vpe, sd, r, tn = (t[:, i * w:(i + 1) * w]
                                             for i in range(6))
                nc.vector.tensor_scalar(mean, stats_sum, 1.0 / count, None,
                                        op0=mybir.AluOpType.mult)
                nc.vector.tensor_scalar(vpe, stats_sq, 1.0 / count, None,
                                        op0=mybir.AluOpType.mult)
                nc.vector.tensor_tensor(msq, mean, mean, op=mybir.AluOpType.mult)
                nc.vector.tensor_tensor(vpe, vpe, msq, op=mybir.AluOpType.subtract)
                nc.vector.tensor_scalar(vpe, vpe, EPS, None, op0=mybir.AluOpType.add)
                nc.scalar.activation(sd, vpe, mybir.ActivationFunctionType.Sqrt)
                nc.vector.reciprocal(r, sd)
                # one Newton step: r *= 1.5 - 0.5*vpe*r*r
                nc.vector.tensor_tensor(tn, r, r, op=mybir.AluOpType.mult)
                nc.vector.tensor_tensor(tn, tn, vpe, op=mybir.AluOpType.mult)
                nc.vector.tensor_scalar(tn, tn, -0.5, 1.5,
                                        op0=mybir.AluOpType.mult,
                                        op1=mybir.AluOpType.add)
                nc.vector.tensor_tensor(r, r, tn, op=mybir.AluOpType.mult)
                co = pool.tile([p, 2 * w], F32, name=f"bnc_{name}")
                scale, shift = co[:, 0:w], co[:, w:2 * w]
                nc.vector.tensor_tensor(scale, g_ap, r, op=mybir.AluOpType.mult)
                nc.vector.tensor_tensor(tn, mean, scale, op=mybir.AluOpType.mult)
                nc.vector.tensor_tensor(shift, b_ap, tn, op=mybir.AluOpType.subtract)
                return scale, shift

            scale1, shift1 = bn_coeffs(
                sp, st1[:, 0:2], st1[:, 2:4], B * P1,
                bsb[:, BC_BN1G:BC_BN1G + 2], bsb[:, BC_BN1B:BC_BN1B + 2], "bn1")
            for mt in range(2):
                nc.vector.tensor_scalar(h1sb[mt][:], h1sb[mt][:],
                                        scale1[:, mt:mt + 1], shift1[:, mt:mt + 1],
                                        op0=mybir.AluOpType.mult,
                                        op1=mybir.AluOpType.add)

            if debug:
                for mt in range(2):
                    dh = sp.tile([128, 1568], F32, name=f"dh{mt}")
                    nc.vector.tensor_copy(dh[:], h1sb[mt][:])
                    nc.sync.dma_start(dbg_h1.ap()[mt], dh[:])
                nc.sync.dma_start(dbg_st1.ap(), st1[:])

            # ---------------- conv2 (strided APs, boundary split) -------
            # psum layout (i2, j2, n): n innermost; two parallel psum chains
            # (one per input-channel block), summed by DVE at the end
            kij_order = [(1, 1), (1, 2), (2, 1), (2, 2), (0, 1), (0, 2),
                         (1, 0), (2, 0), (0, 0)]
            c2ps = []
            for cb2 in range(2):
                cp = c2p.tile([128, P2 * BL], F32, name=f"c2ps{cb2}",
                              tag=f"c2ps{cb2}")
                c2ps.append(cp)
                c2r = cp[:].rearrange("p (i j n) -> p i j n", i=5, j=5, n=BL)
                hr = h1sb[cb2][:].rearrange(
                    "p (n i j) -> p n i j", n=BL, i=14, j=14).transpose([0, 2, 3, 1])
                for cnt, (ki, kj) in enumerate(kij_order):
                    ilo = 1 if ki == 0 else 0
                    jlo = 1 if kj == 0 else 0
                    src = hr[:, 3 * ilo + ki - 1:14:3, 3 * jlo + kj - 1:14:3, :]
                    dst = c2r[:, ilo:, jlo:, :]
                    lhsT = w2sb[:, (cb2 * 9 + ki * 3 + kj) * 128:
                                (cb2 * 9 + ki * 3 + kj + 1) * 128]
                    nc.tensor.matmul(dst, lhsT, src, start=(cnt == 0),
                                     stop=(cnt == 8), skip_group_check=True)
            # DVE has a single PSUM read port: go through SBUF for the add
            c2half = sp.tile([128, BL * P2], F32)
            nc.vector.tensor_copy(c2half[:], c2ps[0][:])
            c2sb = sp.tile([128, BL * P2], BF16)
            nc.vector.tensor_tensor(c2sb[:], c2half[:], c2ps[1][:],
                                    op=mybir.AluOpType.add)

            # ---------------- AllGather conv2 raw -----------------------
            ag_in = dram.tile([128, BL * P2], BF16)
            ag_out = dram.tile([NCORES, 128, BL * P2], BF16, addr_space="Shared")
            nc.scalar.dma_start(ag_in[:], c2sb[:])
            nc.gpsimd.collective_compute(
                "AllGather", mybir.AluOpType.bypass,
                replica_groups=[list(range(NCORES))],
                ins=[ag_in.opt()], outs=[ag_out.opt()])
            g2 = sp.tile([128, B * P2], BF16)
            nc.scalar.dma_start(
                g2[:].rearrange("p (r t) -> p r t", r=NCORES),
                bass.AP(ag_out.tensor, 0,
                        [[BL * P2, 128], [128 * BL * P2, NCORES], [1, BL * P2]]))

            # ---------------- BN2 (redundant, full batch) ---------------
            st2 = sp.tile([128, 2], F32)
            nc.vector.reduce_sum(st2[:, 0:1], g2[:], axis=mybir.AxisListType.X)
            nc.scalar.activation(scratch[:, :B * P2], g2[:],
                                 mybir.ActivationFunctionType.Square,
                                 accum_out=st2[:, 1:2])
            scale2, shift2 = bn_coeffs(
                sp, st2[:, 0:1], st2[:, 1:2], B * P2,
                bsb[:, BC_BN2G:BC_BN2G + 1], bsb[:, BC_BN2B:BC_BN2B + 1], "bn2")
            if debug:
                dg = sp.tile([128, 1600], F32, name="dg")
                nc.vector.tensor_scalar(dg[:], g2[:], scale2, shift2,
                                        op0=mybir.AluOpType.mult,
                                        op1=mybir.AluOpType.add)
                nc.sync.dma_start(dbg_g2.ap(), dg[:])

            # ---------------- collapsed MLP: one matvec + sigmoid -------
            # z[n] = sum_f weff[f]*(g2*s2+t2)[n,f] + beff
            #      = sum_f (weff[f]*s2[c]) * g2[n,f] + sum_c t2[c]*rowsum_weff[c]
            weffn = wp.tile([128, 25], BF16)
            nc.vector.tensor_scalar(weffn[:], weff[:, 0:25], scale2, None,
                                    op0=mybir.AluOpType.mult)
            vsh = wp.tile([128, 1], BF16)
            nc.vector.tensor_tensor(vsh[:], shift2, weff[:, 25:26],
                                    op=mybir.AluOpType.mult)
            ones = wp.tile([128, B], BF16)
            nc.gpsimd.memset(ones[:], 1.0)
            g2v = g2[:].rearrange("p (r i n) -> p r i n", r=NCORES, i=P2)
            zps = zp.tile([1, B], F32)
            for ij in range(P2):
                nc.tensor.matmul(zps[:], weffn[:, ij:ij + 1], g2v[:, :, ij, :],
                                 start=(ij == 0), stop=False)
            nc.tensor.matmul(zps[:], vsh[:], ones[:], start=False, stop=True)
            osb = sp.tile([1, B], F32)
            nc.scalar.activation(osb[:], zps[:],
                                 mybir.ActivationFunctionType.Sigmoid,
                                 bias=bsb[0:1, BC_BEFF:BC_BEFF + 1])
            nc.sync.dma_start(bass.AP(out, 0, [[1, 1], [1, B]]), osb[:])

    nc.compile()
    return nc


# ----------------------------------------------------------------------------
# host-side input prep
# ----------------------------------------------------------------------------

def _prep_inputs(inputs):
    import ml_dtypes
    f = np.float32
    bf = ml_dtypes.bfloat16
    x = np.asarray(inputs["x"], dtype=f)

    # conv1 patches, per core: [4cb, 4pt, 128c, 9kij * 392]
    xpad = np.zeros((B, 512, 42, 42), dtype=bf)
    xpad[:, :, 1:41, 1:41] = x.astype(bf)
    # [n, cb, c, i, ki, j, kj] -> [cb, c, ki, kj, n, i, j]
    xv = xpad.reshape(B, 4, 128, 14, 3, 14, 3).transpose(1, 2, 4, 6, 0, 3, 5)

    w1 = np.asarray(inputs["conv1_w"], dtype=f)          # [256, 512, 3, 3]
    w1p = np.ascontiguousarray(
        w1.reshape(256, 4, 128, 9).transpose(2, 1, 3, 0)).reshape(128, 36, 256).astype(bf)
    w2 = np.asarray(inputs["conv2_w"], dtype=f)          # [128, 256, 3, 3]
    w2p = np.ascontiguousarray(
        w2.reshape(128, 2, 128, 9).transpose(2, 1, 3, 0)).reshape(128, 18, 128).astype(bf)

    # compose the 12 affine layers (no nonlinearities) into [3200] + scalar
    M = np.asarray(inputs["w14"], dtype=np.float64)      # [1, 2]
    beff = np.asarray(inputs["b14"], dtype=np.float64).copy()  # [1]
    for li in range(13, 2, -1):                          # w13 .. w3
        beff += M @ np.asarray(inputs[f"b{li}"], dtype=np.float64)
        M = M @ np.asarray(inputs[f"w{li}"], dtype=np.float64)
    weff = M.reshape(3200).astype(f)                     # order f = c*25 + ij
    w2d = weff.reshape(128, 25)
    weffp = np.zeros((128, 26), dtype=f)
    weffp[:, 0:25] = w2d
    weffp[:, 25] = w2d.sum(axis=1)
    beff_f = float(beff[0])

    bn1_g = np.asarray(inputs["bn1_g"], dtype=f)
    bn1_b = np.asarray(inputs["bn1_b"], dtype=f)
    bn2_g = np.asarray(inputs["bn2_g"], dtype=f)
    bn2_b = np.asarray(inputs["bn2_b"], dtype=f)

    bp = np.zeros((128, 7), dtype=f)
    bp[:, 0:2] = bn1_g.reshape(2, 128).T
    bp[:, 2:4] = bn1_b.reshape(2, 128).T
    bp[:, 4] = bn2_g
    bp[:, 5] = bn2_b
    bp[0, 6] = beff_f

    in_maps = []
    for r in range(NCORES):
        xr = np.ascontiguousarray(
            xv[:, :, :, :, r * BL:(r + 1) * BL]        # [4,128,3,3,8,14,14]
            .reshape(4, 128, 9, NPT, PTW)
            .transpose(0, 3, 1, 2, 4)                  # [4cb, 4pt, 128, 9, 392]
        ).reshape(4, NPT, 128, 9 * PTW)
        in_maps.append({
            "xprep": xr, "w1p": w1p, "w2p": w2p,
            "weffp": weffp, "bprep": bp,
        })
    return in_maps


def kernel(**inputs):
    if "nc" not in _CACHE:
        _CACHE["nc"] = _build()
    nc = _CACHE["nc"]
    in_maps = _prep_inputs(inputs)
    trace = bool(int(os.environ.get("KERNEL_TRACE", "0")))
    if trace:
        import ntff_shim
        ntff_shim.install()
    res = run_bass_kernel_spmd(nc, in_maps, core_ids=list(range(NCORES)),
                               trace=trace)
    _CACHE["last_result"] = res
    return res.results[0]["out"]


# revision 16
# speedup vs baseline: 1.1012x; 1.0868x over previous
"""Trainium2 Bass kernel for nn_DomainDiscriminator.

Network: conv(512->256,k3,s3,p1) -> BN -> conv(256->128,k3,s3,p1) -> BN
         -> reshape -> 12-layer MLP (3200->...->1, no nonlinearities) -> sigmoid.
Input x: [64, 512, 40, 40] f32.  Output: [64, 1] f32.

Strategy (8 NeuronCores):
 - Data-parallel batch shard (8 per core) for the convs.
 - stride==kernel==3 convs are non-overlapping patch matmuls. Conv1 patches are
   built host-side (space-to-depth, free); conv2 patches are read straight out
   of SBUF with strided access patterns (boundary-split matmuls, no im2col).
 - Training-mode BN: conv bias is absorbed exactly by BN; BN1 stats via a 2KB
   AllReduce; BN2 stats computed redundantly after an AllGather of the conv2
   raw output.
 - The 12 linear layers have no activations between them, so they compose on
   the host (fp64) into a single [3200] vector + scalar bias; the device does
   one 25-chunk matvec + sigmoid.
 - Convs run in bf16 (BN re-normalizes, keeping error ~2e-3); the final matvec
   in float32r.
"""

import os
import sys

sys.path.insert(0, "/opt/trn_rl_repo")

import numpy as np

import concourse.bass as bass
import concourse.mybir as mybir
import concourse.tile as tile
from concourse import bacc
from concourse.bass_utils import run_bass_kernel_spmd

F32 = mybir.dt.float32
F32R = mybir.dt.float32r
BF16 = mybir.dt.bfloat16

NCORES = 8
BL = 8              # batch per core
B = 64              # full batch
EPS = 1e-5

# conv1: [BL,512,40,40] -> [BL,256,14,14]; conv2: -> [BL,128,5,5]
P1 = 196            # 14*14 positions
P2 = 25             # 5*5 positions
NPT = 4             # conv1 psum tiles (2 batch each)
PTW = 2 * P1        # 392 columns per conv1 psum tile

_CACHE = {}

KIJ9 = [(ki, kj) for ki in range(3) for kj in range(3)]
# conv2 im2col block offsets within an h1 patch tile [128, 1568]
BLKOFF = {}
_o = 0
for _ki, _kj in KIJ9:
    BLKOFF[(_ki, _kj)] = _o
    _o += (4 if _ki == 0 else 5) * (4 if _kj == 0 else 5) * 8
assert _o == 1568


# ----------------------------------------------------------------------------
# device program
# ----------------------------------------------------------------------------

def _build():
    nc = bacc.Bacc("TRN2", target_bir_lowering=False, debug=False,
                   enable_asserts=True, num_devices=NCORES)

    xprep = nc.dram_tensor("xprep", [4, NPT, 128, 9 * PTW], BF16,
                           kind="ExternalInput")
    w1p = nc.dram_tensor("w1p", [128, 36, 256], BF16, kind="ExternalInput")
    w2p = nc.dram_tensor("w2p", [128, 18, 128], BF16, kind="ExternalInput")
    weffp = nc.dram_tensor("weffp", [128, 26], F32, kind="ExternalInput")
    bprep = nc.dram_tensor("bprep", [128, 7], F32, kind="ExternalInput")
    out = nc.dram_tensor("out", [B, 1], F32, kind="ExternalOutput")
    debug = bool(int(os.environ.get("KERNEL_DEBUG", "0")))
    if debug:
        dbg_h1 = nc.dram_tensor("dbg_h1", [2, 128, 1568], F32, kind="ExternalOutput")
        dbg_g2 = nc.dram_tensor("dbg_g2", [128, 1600], F32, kind="ExternalOutput")
        dbg_st1 = nc.dram_tensor("dbg_st1", [128, 4], F32, kind="ExternalOutput")

    # bprep columns: bn1_g (2), bn1_b (2), bn2_g, bn2_b, beff(row 0)
    BC_BN1G, BC_BN1B, BC_BN2G, BC_BN2B, BC_BEFF = 0, 2, 4, 5, 6

    with tile.TileContext(nc) as tc:
        with tc.tile_pool(name="wp", bufs=1) as wp, \
             tc.tile_pool(name="xp", bufs=4) as xp, \
             tc.tile_pool(name="hp", bufs=1) as hp, \
             tc.tile_pool(name="sp", bufs=1) as sp, \
             tc.tile_pool(name="cps", bufs=4, space="PSUM") as cps, \
             tc.tile_pool(name="c2p", bufs=1, space="PSUM") as c2p, \
             tc.tile_pool(name="zp", bufs=1, space="PSUM") as zp, \
             tc.tile_pool(name="dram", bufs=1, space="DRAM") as dram:

            # ---------------- weight/bias loads -------------------------
            w1sb = wp.tile([128, 36 * 256], BF16)
            w1r = w1p.ap().rearrange("p a b -> p (a b)")
            nc.sync.dma_start(w1sb[:, 0:9 * 256], w1r[:, 0:9 * 256])

            # ncfw warm-up: a tiny AllGather nobody consumes; hides the
            # ~12us TOPSP cold-start under conv1
            warm_in = dram.tile([1, 4], F32)
            warm_out = dram.tile([NCORES, 1, 4], F32, addr_space="Shared")
            dummy = sp.tile([1, 4], F32)
            nc.gpsimd.memset(dummy[:], 0.0)
            nc.scalar.dma_start(warm_in[:], dummy[:])
            nc.gpsimd.collective_compute(
                "AllGather", mybir.AluOpType.bypass,
                replica_groups=[list(range(NCORES))],
                ins=[warm_in.opt()], outs=[warm_out.opt()])
            # ACT Square table preload while ACT is idle
            nc.scalar.activation(dummy[:, 0:1], dummy[:, 1:2],
                                 mybir.ActivationFunctionType.Square)

            # ---------------- conv1 -------------------------------------
            h1sb = [hp.tile([128, 4 * PTW], BF16, name=f"h1_{mt}") for mt in range(2)]
            for pt in range(NPT):
                ps = [cps.tile([128, PTW], F32, name="c1ps", tag="c1ps")
                      for _ in range(2)]
                for cb in range(4):
                    xt = xp.tile([128, 9 * PTW], BF16, name="xt", tag="xt")
                    nc.sync.dma_start(xt[:], xprep.ap()[cb, pt])
                    if pt == 0 and cb < 3:
                        # stream the rest of w1 behind the first x chunk
                        sl = slice((cb + 1) * 9 * 256, (cb + 2) * 9 * 256)
                        nc.sync.dma_start(w1sb[:, sl], w1r[:, sl])
                    xtr = xt[:].rearrange("p (k c) -> p k c", k=9)
                    for kij in range(9):
                        rhs = xtr[:, kij]
                        for mt in range(2):
                            lhsT = w1sb[:, (cb * 9 + kij) * 256 + mt * 128:
                                        (cb * 9 + kij) * 256 + (mt + 1) * 128]
                            nc.tensor.matmul(ps[mt][:], lhsT, rhs,
                                             start=(cb == 0 and kij == 0),
                                             stop=(cb == 3 and kij == 8))
                for mt in range(2):
                    pr = ps[mt][:].rearrange("p (n i j) -> p n i j",
                                             n=2, i=14, j=14)
                    for (ki, kj) in KIJ9:
                        ilo, icnt = (1, 4) if ki == 0 else (0, 5)
                        jlo, jcnt = (1, 4) if kj == 0 else (0, 5)
                        srcv = pr[:, :, 3 * ilo + ki - 1:14:3,
                                  3 * jlo + kj - 1:14:3].transpose([0, 2, 3, 1])
                        off = BLKOFF[(ki, kj)]
                        dstv = bass.AP(
                            h1sb[mt].tensor, h1sb[mt].offset + off + 2 * pt,
                            [list(h1sb[mt].ap[0]), [jcnt * 8, icnt], [8, jcnt],
                             [1, 2]])
                        nc.vector.tensor_copy(dstv, srcv)

            # late loads (behind the x stream on the SP ring)
            w2sb = wp.tile([128, 18 * 128], BF16)
            nc.sync.dma_start(w2sb[:], w2p.ap().rearrange("p a b -> p (a b)"))
            weff = wp.tile([128, 26], F32)
            nc.sync.dma_start(weff[:], weffp.ap())
            bsb = wp.tile([128, 7], F32)
            nc.sync.dma_start(bsb[:], bprep.ap())

            # ---------------- BN1 stats + AllReduce ---------------------
            # bounce DMAs ride the Scalar HWDGE ring so they are not stuck
            # behind bulk loads on the SP ring
            scratch = sp.tile([128, 1600], F32)
            st_in = sp.tile([128, 4], F32)
            for mt in range(2):
                h = h1sb[mt][:]
                nc.vector.reduce_sum(st_in[:, mt:mt + 1], h,
                                     axis=mybir.AxisListType.X)
                nc.scalar.activation(scratch[:, :4 * PTW], h,
                                     mybir.ActivationFunctionType.Square,
                                     accum_out=st_in[:, 2 + mt:3 + mt])
            # preload Sqrt/Sigmoid tables while waiting for the AllGather
            nc.scalar.activation(dummy[:, 0:1], dummy[:, 1:2],
                                 mybir.ActivationFunctionType.Sqrt)
            nc.scalar.activation(dummy[:, 0:1], dummy[:, 1:2],
                                 mybir.ActivationFunctionType.Sigmoid)
            bn1_in = dram.tile([128, 4], F32)
            bn1_out = dram.tile([NCORES, 128, 4], F32, addr_space="Shared")
            nc.scalar.dma_start(bn1_in[:], st_in[:])
            nc.gpsimd.collective_compute(
                "AllGather", mybir.AluOpType.bypass,
                replica_groups=[list(range(NCORES))],
                ins=[bn1_in.opt()], outs=[bn1_out.opt()])
            stg = sp.tile([128, NCORES * 4], F32)
            nc.scalar.dma_start(
                stg[:].rearrange("p (r t) -> p r t", r=NCORES),
                bass.AP(bn1_out.tensor, 0, [[4, 128], [128 * 4, NCORES], [1, 4]]))
            # tree-sum the 8 rank blocks: 8x4 -> 4x4 -> 2x4 -> 1x4
            stgr = stg[:].rearrange("p (r t) -> p r t", r=NCORES)
            for half in (4, 2, 1):
                nc.vector.tensor_tensor(
                    stgr[:, 0:half], stgr[:, 0:half], stgr[:, half:2 * half],
                    op=mybir.AluOpType.add)
            st1 = stg[:, 0:4]

            # ---------------- BN1 scale/shift + apply -------------------
            def bn_coeffs(pool, stats_sum, stats_sq, count, g_ap, b_ap, name):
                """returns (scale, shift) [p,w] tiles; stats_* are [p,w] APs"""
                p, w = stats_sum.shape
                t = pool.tile([p, 6 * w], F32, name=f"bn_{name}")
                mean, msq, vpe, sd, r, tn = (t[:, i * w:(i + 1) * w]
                                             for i in range(6))
                nc.vector.tensor_scalar(mean, stats_sum, 1.0 / count, None,
                                        op0=mybir.AluOpType.mult)
                nc.vector.tensor_scalar(vpe, stats_sq, 1.0 / count, None,
                                        op0=mybir.AluOpType.mult)
                nc.vector.tensor_tensor(msq, mean, mean, op=mybir.AluOpType.mult)
                nc.vector.tensor_tensor(vpe, vpe, msq, op=mybir.AluOpType.subtract)
                nc.vector.tensor_scalar(vpe, vpe, EPS, None, op0=mybir.AluOpType.add)
                nc.scalar.activation(sd, vpe, mybir.ActivationFunctionType.Sqrt)
                nc.vector.reciprocal(r, sd)
                co = pool.tile([p, 2 * w], F32, name=f"bnc_{name}")
                scale, shift = co[:, 0:w], co[:, w:2 * w]
                nc.vector.tensor_tensor(scale, g_ap, r, op=mybir.AluOpType.mult)
                nc.vector.tensor_tensor(tn, mean, scale, op=mybir.AluOpType.mult)
                nc.vector.tensor_tensor(shift, b_ap, tn, op=mybir.AluOpType.subtract)
                return scale, shift

            scale1, shift1 = bn_coeffs(
                sp, st1[:, 0:2], st1[:, 2:4], B * P1,
                bsb[:, BC_BN1G:BC_BN1G + 2], bsb[:, BC_BN1B:BC_BN1B + 2], "bn1")
            for mt in range(2):
                nc.vector.tensor_scalar(h1sb[mt][:], h1sb[mt][:],
                                        scale1[:, mt:mt + 1], shift1[:, mt:mt + 1],
                                        op0=mybir.AluOpType.mult,
                                        op1=mybir.AluOpType.add)

            if debug:
                for mt in range(2):
                    dh = sp.tile([128, 1568], F32, name=f"dh{mt}")
                    nc.vector.tensor_copy(dh[:], h1sb[mt][:])
                    nc.sync.dma_start(dbg_h1.ap()[mt], dh[:])
                nc.sync.dma_start(dbg_st1.ap(), st1[:])

            # ---------------- conv2 (contiguous im2col blocks) ----------
            # psum layout (i2, j2, n): n innermost; two parallel psum chains
            # (one per input-channel block), summed by DVE at the end
            kij_order = [(1, 1), (1, 2), (2, 1), (2, 2), (0, 1), (0, 2),
                         (1, 0), (2, 0), (0, 0)]
            c2ps = []
            for cb2 in range(2):
                cp = c2p.tile([128, P2 * BL], F32, name=f"c2ps{cb2}",
                              tag=f"c2ps{cb2}")
                c2ps.append(cp)
                c2r = cp[:].rearrange("p (i j n) -> p i j n", i=5, j=5, n=BL)
                for cnt, (ki, kj) in enumerate(kij_order):
                    ilo, icnt = (1, 4) if ki == 0 else (0, 5)
                    jlo, jcnt = (1, 4) if kj == 0 else (0, 5)
                    off = BLKOFF[(ki, kj)]
                    src = h1sb[cb2][:, off:off + icnt * jcnt * 8]
                    dst = c2r[:, ilo:, jlo:, :]
                    lhsT = w2sb[:, (cb2 * 9 + ki * 3 + kj) * 128:
                                (cb2 * 9 + ki * 3 + kj + 1) * 128]
                    nc.tensor.matmul(dst, lhsT, src, start=(cnt == 0),
                                     stop=(cnt == 8), skip_group_check=True)
            # DVE has a single PSUM read port: go through SBUF for the add
            c2half = sp.tile([128, BL * P2], F32)
            nc.vector.tensor_copy(c2half[:], c2ps[0][:])
            c2sb = sp.tile([128, BL * P2], BF16)
            nc.vector.tensor_tensor(c2sb[:], c2half[:], c2ps[1][:],
                                    op=mybir.AluOpType.add)

            # ---------------- AllGather conv2 raw -----------------------
            ag_in = dram.tile([128, BL * P2], BF16)
            ag_out = dram.tile([NCORES, 128, BL * P2], BF16, addr_space="Shared")
            nc.scalar.dma_start(ag_in[:], c2sb[:])
            nc.gpsimd.collective_compute(
                "AllGather", mybir.AluOpType.bypass,
                replica_groups=[list(range(NCORES))],
                ins=[ag_in.opt()], outs=[ag_out.opt()])
            g2 = sp.tile([128, B * P2], BF16)
            nc.scalar.dma_start(
                g2[:].rearrange("p (r t) -> p r t", r=NCORES),
                bass.AP(ag_out.tensor, 0,
                        [[BL * P2, 128], [128 * BL * P2, NCORES], [1, BL * P2]]))

            # ---------------- BN2 (redundant, full batch) ---------------
            st2 = sp.tile([128, 2], F32)
            nc.vector.reduce_sum(st2[:, 0:1], g2[:], axis=mybir.AxisListType.X)
            nc.scalar.activation(scratch[:, :B * P2], g2[:],
                                 mybir.ActivationFunctionType.Square,
                                 accum_out=st2[:, 1:2])
            scale2, shift2 = bn_coeffs(
                sp, st2[:, 0:1], st2[:, 1:2], B * P2,
                bsb[:, BC_BN2G:BC_BN2G + 1], bsb[:, BC_BN2B:BC_BN2B + 1], "bn2")
            if debug:
                dg = sp.tile([128, 1600], F32, name="dg")
                nc.vector.tensor_scalar(dg[:], g2[:], scale2, shift2,
                                        op0=mybir.AluOpType.mult,
                                        op1=mybir.AluOpType.add)
                nc.sync.dma_start(dbg_g2.ap(), dg[:])

            # ---------------- collapsed MLP: one matvec + sigmoid -------
            # z[n] = sum_f weff[f]*(g2*s2+t2)[n,f] + beff
            #      = sum_f (weff[f]*s2[c]) * g2[n,f] + sum_c t2[c]*rowsum_weff[c]
            weffn = wp.tile([128, 25], BF16)
            nc.vector.tensor_scalar(weffn[:], weff[:, 0:25], scale2, None,
                                    op0=mybir.AluOpType.mult)
            vsh = wp.tile([128, 1], BF16)
            nc.vector.tensor_tensor(vsh[:], shift2, weff[:, 25:26],
                                    op=mybir.AluOpType.mult)
            ones = wp.tile([128, B], BF16)
            nc.gpsimd.memset(ones[:], 1.0)
            g2v = g2[:].rearrange("p (r i n) -> p r i n", r=NCORES, i=P2)
            zps = zp.tile([1, B], F32)
            for ij in range(P2):
                nc.tensor.matmul(zps[:], weffn[:, ij:ij + 1], g2v[:, :, ij, :],
                                 start=(ij == 0), stop=False)
            nc.tensor.matmul(zps[:], vsh[:], ones[:], start=False, stop=True)
            osb = sp.tile([1, B], F32)
            nc.scalar.activation(osb[:], zps[:],
                                 mybir.ActivationFunctionType.Sigmoid,
                                 bias=bsb[0:1, BC_BEFF:BC_BEFF + 1])
            nc.sync.dma_start(bass.AP(out, 0, [[1, 1], [1, B]]), osb[:])

    nc.compile()
    return nc


# ----------------------------------------------------------------------------
# host-side input prep
# ----------------------------------------------------------------------------

def _prep_inputs(inputs):
    import ml_dtypes
    f = np.float32
    bf = ml_dtypes.bfloat16
    x = np.asarray(inputs["x"], dtype=f)

    # conv1 patches, per core: [4cb, 4pt, 128c, 9kij * 392]
    xpad = np.zeros((B, 512, 42, 42), dtype=bf)
    xpad[:, :, 1:41, 1:41] = x.astype(bf)
    # [n, cb, c, i, ki, j, kj] -> [cb, c, ki, kj, n, i, j]
    xv = xpad.reshape(B, 4, 128, 14, 3, 14, 3).transpose(1, 2, 4, 6, 0, 3, 5)

    w1 = np.asarray(inputs["conv1_w"], dtype=f)          # [256, 512, 3, 3]
    w1p = np.ascontiguousarray(
        w1.reshape(256, 4, 128, 9).transpose(2, 1, 3, 0)).reshape(128, 36, 256).astype(bf)
    w2 = np.asarray(inputs["conv2_w"], dtype=f)          # [128, 256, 3, 3]
    w2p = np.ascontiguousarray(
        w2.reshape(128, 2, 128, 9).transpose(2, 1, 3, 0)).reshape(128, 18, 128).astype(bf)

    # compose the 12 affine layers (no nonlinearities) into [3200] + scalar
    M = np.asarray(inputs["w14"], dtype=np.float64)      # [1, 2]
    beff = np.asarray(inputs["b14"], dtype=np.float64).copy()  # [1]
    for li in range(13, 2, -1):                          # w13 .. w3
        beff += M @ np.asarray(inputs[f"b{li}"], dtype=np.float64)
        M = M @ np.asarray(inputs[f"w{li}"], dtype=np.float64)
    weff = M.reshape(3200).astype(f)                     # order f = c*25 + ij
    w2d = weff.reshape(128, 25)
    weffp = np.zeros((128, 26), dtype=f)
    weffp[:, 0:25] = w2d
    weffp[:, 25] = w2d.sum(axis=1)
    beff_f = float(beff[0])

    bn1_g = np.asarray(inputs["bn1_g"], dtype=f)
    bn1_b = np.asarray(inputs["bn1_b"], dtype=f)
    bn2_g = np.asarray(inputs["bn2_g"], dtype=f)
    bn2_b = np.asarray(inputs["bn2_b"], dtype=f)

    bp = np.zeros((128, 7), dtype=f)
    bp[:, 0:2] = bn1_g.reshape(2, 128).T
    bp[:, 2:4] = bn1_b.reshape(2, 128).T
    bp[:, 4] = bn2_g
    bp[:, 5] = bn2_b
    bp[0, 6] = beff_f

    in_maps = []
    for r in range(NCORES):
        xr = np.ascontiguousarray(
            xv[:, :, :, :, r * BL:(r + 1) * BL]        # [4,128,3,3,8,14,14]
            .reshape(4, 128, 9, NPT, PTW)
            .transpose(0, 3, 1, 2, 4)                  # [4cb, 4pt, 128, 9, 392]
        ).reshape(4, NPT, 128, 9 * PTW)
        in_maps.append({
            "xprep": xr, "w1p": w1p, "w2p": w2p,
            "weffp": weffp, "bprep": bp,
        })
    return in_maps


def kernel(**inputs):
    if "nc" not in _CACHE:
        _CACHE["nc"] = _build()
    nc = _CACHE["nc"]
    in_maps = _prep_inputs(inputs)
    trace = bool(int(os.environ.get("KERNEL_TRACE", "0")))
    if trace:
        import ntff_shim
        ntff_shim.install()
    res = run_bass_kernel_spmd(nc, in_maps, core_ids=list(range(NCORES)),
                               trace=trace)
    _CACHE["last_result"] = res
    return res.results[0]["out"]


# revision 17
# speedup vs baseline: 1.2880x; 1.1696x over previous
"""Trainium2 Bass kernel for nn_DomainDiscriminator.

Network: conv(512->256,k3,s3,p1) -> BN -> conv(256->128,k3,s3,p1) -> BN
         -> reshape -> 12-layer MLP (3200->...->1, no nonlinearities) -> sigmoid.
Input x: [64, 512, 40, 40] f32.  Output: [64, 1] f32.

Strategy (8 NeuronCores):
 - Data-parallel batch shard (8 per core) for the convs.
 - stride==kernel==3 convs are non-overlapping patch matmuls. Conv1 patches are
   built host-side (space-to-depth, free); conv2 patches are read straight out
   of SBUF with strided access patterns (boundary-split matmuls, no im2col).
 - Training-mode BN: conv bias is absorbed exactly by BN; BN1 stats via a 2KB
   AllReduce; BN2 stats computed redundantly after an AllGather of the conv2
   raw output.
 - The 12 linear layers have no activations between them, so they compose on
   the host (fp64) into a single [3200] vector + scalar bias; the device does
   one 25-chunk matvec + sigmoid.
 - Convs run in bf16 (BN re-normalizes, keeping error ~2e-3); the final matvec
   in float32r.
"""

import os
import sys

sys.path.insert(0, "/opt/trn_rl_repo")

import numpy as np

import concourse.bass as bass
import concourse.mybir as mybir
import concourse.tile as tile
from concourse import bacc
from concourse.bass_utils import run_bass_kernel_spmd

F32 = mybir.dt.float32
F32R = mybir.dt.float32r
BF16 = mybir.dt.bfloat16

NCORES = 8
BL = 8              # batch per core
B = 64              # full batch
EPS = 1e-5

# conv1: [BL,512,40,40] -> [BL,256,14,14]; conv2: -> [BL,128,5,5]
P1 = 196            # 14*14 positions
P2 = 25             # 5*5 positions
NPT = 4             # conv1 psum tiles (2 batch each)
PTW = 2 * P1        # 392 columns per conv1 psum tile

_CACHE = {}

KIJ9 = [(ki, kj) for ki in range(3) for kj in range(3)]
# conv2 im2col block offsets within an h1 patch tile [128, 1568]
BLKOFF = {}
_o = 0
for _ki, _kj in KIJ9:
    BLKOFF[(_ki, _kj)] = _o
    _o += (4 if _ki == 0 else 5) * (4 if _kj == 0 else 5) * 8
assert _o == 1568


# ----------------------------------------------------------------------------
# device program
# ----------------------------------------------------------------------------

def _build():
    nc = bacc.Bacc("TRN2", target_bir_lowering=False, debug=False,
                   enable_asserts=True, num_devices=NCORES)

    xprep = nc.dram_tensor("xprep", [4, NPT, 128, 9 * PTW], BF16,
                           kind="ExternalInput")
    w1p = nc.dram_tensor("w1p", [128, 36, 256], BF16, kind="ExternalInput")
    w2p = nc.dram_tensor("w2p", [128, 18, 128], BF16, kind="ExternalInput")
    weffp = nc.dram_tensor("weffp", [128, 26], F32, kind="ExternalInput")
    bprep = nc.dram_tensor("bprep", [128, 7], F32, kind="ExternalInput")
    out = nc.dram_tensor("out", [BL, 1], F32, kind="ExternalOutput")
    debug = bool(int(os.environ.get("KERNEL_DEBUG", "0")))
    if debug:
        dbg_h1 = nc.dram_tensor("dbg_h1", [2, 128, 1568], F32, kind="ExternalOutput")
        dbg_st1 = nc.dram_tensor("dbg_st1", [128, 4], F32, kind="ExternalOutput")

    # bprep columns: bn1_g (2), bn1_b (2), bn2_g, bn2_b, beff(row 0)
    BC_BN1G, BC_BN1B, BC_BN2G, BC_BN2B, BC_BEFF = 0, 2, 4, 5, 6

    with tile.TileContext(nc) as tc:
        with tc.tile_pool(name="wp", bufs=1) as wp, \
             tc.tile_pool(name="xp", bufs=4) as xp, \
             tc.tile_pool(name="hp", bufs=1) as hp, \
             tc.tile_pool(name="sp", bufs=1) as sp, \
             tc.tile_pool(name="cps", bufs=4, space="PSUM") as cps, \
             tc.tile_pool(name="c2p", bufs=1, space="PSUM") as c2p, \
             tc.tile_pool(name="zp", bufs=1, space="PSUM") as zp, \
             tc.tile_pool(name="dram", bufs=1, space="DRAM") as dram:

            # ---------------- weight/bias loads -------------------------
            w1sb = wp.tile([128, 36 * 256], BF16)
            w1r = w1p.ap().rearrange("p a b -> p (a b)")
            nc.sync.dma_start(w1sb[:, 0:9 * 256], w1r[:, 0:9 * 256])

            # ncfw warm-up: a tiny AllGather nobody consumes; hides the
            # ~12us TOPSP cold-start under conv1
            warm_in = dram.tile([1, 4], F32)
            warm_out = dram.tile([NCORES, 1, 4], F32, addr_space="Shared")
            dummy = sp.tile([1, 4], F32)
            nc.gpsimd.memset(dummy[:], 0.0)
            nc.scalar.dma_start(warm_in[:], dummy[:])
            nc.gpsimd.collective_compute(
                "AllGather", mybir.AluOpType.bypass,
                replica_groups=[list(range(NCORES))],
                ins=[warm_in.opt()], outs=[warm_out.opt()])
            # ACT Square table preload while ACT is idle
            nc.scalar.activation(dummy[:, 0:1], dummy[:, 1:2],
                                 mybir.ActivationFunctionType.Square)

            # ---------------- conv1 -------------------------------------
            h1sb = [hp.tile([128, 4 * PTW], BF16, name=f"h1_{mt}") for mt in range(2)]
            for pt in range(NPT):
                ps = [cps.tile([128, PTW], F32, name="c1ps", tag="c1ps")
                      for _ in range(2)]
                for cb in range(4):
                    xt = xp.tile([128, 9 * PTW], BF16, name="xt", tag="xt")
                    nc.sync.dma_start(xt[:], xprep.ap()[cb, pt])
                    if pt == 0 and cb < 3:
                        # stream the rest of w1 behind the first x chunk
                        sl = slice((cb + 1) * 9 * 256, (cb + 2) * 9 * 256)
                        nc.sync.dma_start(w1sb[:, sl], w1r[:, sl])
                    xtr = xt[:].rearrange("p (k c) -> p k c", k=9)
                    for kij in range(9):
                        rhs = xtr[:, kij]
                        for mt in range(2):
                            lhsT = w1sb[:, (cb * 9 + kij) * 256 + mt * 128:
                                        (cb * 9 + kij) * 256 + (mt + 1) * 128]
                            nc.tensor.matmul(ps[mt][:], lhsT, rhs,
                                             start=(cb == 0 and kij == 0),
                                             stop=(cb == 3 and kij == 8))
                for mt in range(2):
                    pr = ps[mt][:].rearrange("p (n i j) -> p n i j",
                                             n=2, i=14, j=14)
                    for (ki, kj) in KIJ9:
                        ilo, icnt = (1, 4) if ki == 0 else (0, 5)
                        jlo, jcnt = (1, 4) if kj == 0 else (0, 5)
                        srcv = pr[:, :, 3 * ilo + ki - 1:14:3,
                                  3 * jlo + kj - 1:14:3].transpose([0, 2, 3, 1])
                        off = BLKOFF[(ki, kj)]
                        dstv = bass.AP(
                            h1sb[mt].tensor, h1sb[mt].offset + off + 2 * pt,
                            [list(h1sb[mt].ap[0]), [jcnt * 8, icnt], [8, jcnt],
                             [1, 2]])
                        nc.vector.tensor_copy(dstv, srcv)

            # late loads (behind the x stream on the SP ring)
            w2sb = wp.tile([128, 18 * 128], BF16)
            nc.sync.dma_start(w2sb[:], w2p.ap().rearrange("p a b -> p (a b)"))
            weff = wp.tile([128, 26], F32)
            nc.sync.dma_start(weff[:], weffp.ap())
            bsb = wp.tile([128, 7], F32)
            nc.sync.dma_start(bsb[:], bprep.ap())

            # ---------------- BN1 stats + AllReduce ---------------------
            # bounce DMAs ride the Scalar HWDGE ring so they are not stuck
            # behind bulk loads on the SP ring
            scratch = sp.tile([128, 1600], F32)
            st_in = sp.tile([128, 4], F32)
            for mt in range(2):
                h = h1sb[mt][:]
                nc.vector.reduce_sum(st_in[:, mt:mt + 1], h,
                                     axis=mybir.AxisListType.X)
                nc.scalar.activation(scratch[:, :4 * PTW], h,
                                     mybir.ActivationFunctionType.Square,
                                     accum_out=st_in[:, 2 + mt:3 + mt])
            # preload Sqrt/Sigmoid tables while waiting for the AllGather
            nc.scalar.activation(dummy[:, 0:1], dummy[:, 1:2],
                                 mybir.ActivationFunctionType.Sqrt)
            nc.scalar.activation(dummy[:, 0:1], dummy[:, 1:2],
                                 mybir.ActivationFunctionType.Sigmoid)
            bn1_in = dram.tile([128, 4], F32)
            bn1_out = dram.tile([NCORES, 128, 4], F32, addr_space="Shared")
            nc.scalar.dma_start(bn1_in[:], st_in[:])
            nc.gpsimd.collective_compute(
                "AllGather", mybir.AluOpType.bypass,
                replica_groups=[list(range(NCORES))],
                ins=[bn1_in.opt()], outs=[bn1_out.opt()])
            stg = sp.tile([128, NCORES * 4], F32)
            nc.scalar.dma_start(
                stg[:].rearrange("p (r t) -> p r t", r=NCORES),
                bass.AP(bn1_out.tensor, 0, [[4, 128], [128 * 4, NCORES], [1, 4]]))
            # tree-sum the 8 rank blocks: 8x4 -> 4x4 -> 2x4 -> 1x4
            stgr = stg[:].rearrange("p (r t) -> p r t", r=NCORES)
            for half in (4, 2, 1):
                nc.vector.tensor_tensor(
                    stgr[:, 0:half], stgr[:, 0:half], stgr[:, half:2 * half],
                    op=mybir.AluOpType.add)
            st1 = stg[:, 0:4]

            # ---------------- BN1 scale/shift + apply -------------------
            def bn_coeffs(pool, stats_sum, stats_sq, count, g_ap, b_ap, name):
                """returns (scale, shift) [p,w] tiles; stats_* are [p,w] APs"""
                p, w = stats_sum.shape
                t = pool.tile([p, 6 * w], F32, name=f"bn_{name}")
                mean, msq, vpe, sd, r, tn = (t[:, i * w:(i + 1) * w]
                                             for i in range(6))
                nc.vector.tensor_scalar(mean, stats_sum, 1.0 / count, None,
                                        op0=mybir.AluOpType.mult)
                nc.vector.tensor_scalar(vpe, stats_sq, 1.0 / count, None,
                                        op0=mybir.AluOpType.mult)
                nc.vector.tensor_tensor(msq, mean, mean, op=mybir.AluOpType.mult)
                nc.vector.tensor_tensor(vpe, vpe, msq, op=mybir.AluOpType.subtract)
                nc.vector.tensor_scalar(vpe, vpe, EPS, None, op0=mybir.AluOpType.add)
                nc.scalar.activation(sd, vpe, mybir.ActivationFunctionType.Sqrt)
                nc.vector.reciprocal(r, sd)
                co = pool.tile([p, 2 * w], F32, name=f"bnc_{name}")
                scale, shift = co[:, 0:w], co[:, w:2 * w]
                nc.vector.tensor_tensor(scale, g_ap, r, op=mybir.AluOpType.mult)
                nc.vector.tensor_tensor(tn, mean, scale, op=mybir.AluOpType.mult)
                nc.vector.tensor_tensor(shift, b_ap, tn, op=mybir.AluOpType.subtract)
                return scale, shift

            scale1, shift1 = bn_coeffs(
                sp, st1[:, 0:2], st1[:, 2:4], B * P1,
                bsb[:, BC_BN1G:BC_BN1G + 2], bsb[:, BC_BN1B:BC_BN1B + 2], "bn1")
            for mt in range(2):
                nc.vector.tensor_scalar(h1sb[mt][:], h1sb[mt][:],
                                        scale1[:, mt:mt + 1], shift1[:, mt:mt + 1],
                                        op0=mybir.AluOpType.mult,
                                        op1=mybir.AluOpType.add)

            if debug:
                for mt in range(2):
                    dh = sp.tile([128, 1568], F32, name=f"dh{mt}")
                    nc.vector.tensor_copy(dh[:], h1sb[mt][:])
                    nc.sync.dma_start(dbg_h1.ap()[mt], dh[:])
                nc.sync.dma_start(dbg_st1.ap(), st1[:])

            # ---------------- conv2 (contiguous im2col blocks) ----------
            # psum layout (i2, j2, n): n innermost; two parallel psum chains
            # (one per input-channel block), summed by DVE at the end
            kij_order = [(1, 1), (1, 2), (2, 1), (2, 2), (0, 1), (0, 2),
                         (1, 0), (2, 0), (0, 0)]
            c2ps = []
            for cb2 in range(2):
                cp = c2p.tile([128, P2 * BL], F32, name=f"c2ps{cb2}",
                              tag=f"c2ps{cb2}")
                c2ps.append(cp)
                c2r = cp[:].rearrange("p (i j n) -> p i j n", i=5, j=5, n=BL)
                for cnt, (ki, kj) in enumerate(kij_order):
                    ilo, icnt = (1, 4) if ki == 0 else (0, 5)
                    jlo, jcnt = (1, 4) if kj == 0 else (0, 5)
                    off = BLKOFF[(ki, kj)]
                    src = h1sb[cb2][:, off:off + icnt * jcnt * 8]
                    dst = c2r[:, ilo:, jlo:, :]
                    lhsT = w2sb[:, (cb2 * 9 + ki * 3 + kj) * 128:
                                (cb2 * 9 + ki * 3 + kj + 1) * 128]
                    nc.tensor.matmul(dst, lhsT, src, start=(cnt == 0),
                                     stop=(cnt == 8), skip_group_check=True)
            # DVE has a single PSUM read port: go through SBUF for the add
            c2half = sp.tile([128, BL * P2], F32)
            nc.vector.tensor_copy(c2half[:], c2ps[0][:])
            c2sb = sp.tile([128, BL * P2], BF16)
            nc.vector.tensor_tensor(c2sb[:], c2half[:], c2ps[1][:],
                                    op=mybir.AluOpType.add)

            # ---------------- BN2 stats exchange (1KB AllGather) --------
            st2l = sp.tile([128, 2], F32)
            nc.vector.reduce_sum(st2l[:, 0:1], c2sb[:], axis=mybir.AxisListType.X)
            nc.scalar.activation(scratch[:, :BL * P2], c2sb[:],
                                 mybir.ActivationFunctionType.Square,
                                 accum_out=st2l[:, 1:2])
            bn2_in = dram.tile([128, 2], F32)
            bn2_out = dram.tile([NCORES, 128, 2], F32, addr_space="Shared")
            nc.scalar.dma_start(bn2_in[:], st2l[:])
            nc.gpsimd.collective_compute(
                "AllGather", mybir.AluOpType.bypass,
                replica_groups=[list(range(NCORES))],
                ins=[bn2_in.opt()], outs=[bn2_out.opt()])
            stg2 = sp.tile([128, NCORES * 2], F32)
            nc.scalar.dma_start(
                stg2[:].rearrange("p (r t) -> p r t", r=NCORES),
                bass.AP(bn2_out.tensor, 0, [[2, 128], [128 * 2, NCORES], [1, 2]]))
            stg2r = stg2[:].rearrange("p (r t) -> p r t", r=NCORES)
            for half in (4, 2, 1):
                nc.vector.tensor_tensor(
                    stg2r[:, 0:half], stg2r[:, 0:half], stg2r[:, half:2 * half],
                    op=mybir.AluOpType.add)
            scale2, shift2 = bn_coeffs(
                sp, stg2[:, 0:1], stg2[:, 1:2], B * P2,
                bsb[:, BC_BN2G:BC_BN2G + 1], bsb[:, BC_BN2B:BC_BN2B + 1], "bn2")

            # ---------------- collapsed MLP on the local batch ----------
            # z[n] = sum_f (weff[f]*s2[c]) * c2[n,f] + sum_c t2[c]*rowsum_weff[c]
            weffn = wp.tile([128, 25], BF16)
            nc.vector.tensor_scalar(weffn[:], weff[:, 0:25], scale2, None,
                                    op0=mybir.AluOpType.mult)
            vsh = wp.tile([128, 1], BF16)
            nc.vector.tensor_tensor(vsh[:], shift2, weff[:, 25:26],
                                    op=mybir.AluOpType.mult)
            ones = wp.tile([128, BL], BF16)
            nc.gpsimd.memset(ones[:], 1.0)
            zps = zp.tile([1, BL], F32)
            for ij in range(P2):
                nc.tensor.matmul(zps[:], weffn[:, ij:ij + 1],
                                 c2sb[:, ij * BL:(ij + 1) * BL],
                                 start=(ij == 0), stop=False)
            nc.tensor.matmul(zps[:], vsh[:], ones[:], start=False, stop=True)
            osb = sp.tile([1, BL], F32)
            nc.scalar.activation(osb[:], zps[:],
                                 mybir.ActivationFunctionType.Sigmoid,
                                 bias=bsb[0:1, BC_BEFF:BC_BEFF + 1])
            nc.sync.dma_start(bass.AP(out, 0, [[1, 1], [1, BL]]), osb[:])

    nc.compile()
    return nc


# ----------------------------------------------------------------------------
# host-side input prep
# ----------------------------------------------------------------------------

def _prep_inputs(inputs):
    import ml_dtypes
    f = np.float32
    bf = ml_dtypes.bfloat16
    x = np.asarray(inputs["x"], dtype=f)

    # conv1 patches, per core: [4cb, 4pt, 128c, 9kij * 392]
    xpad = np.zeros((B, 512, 42, 42), dtype=bf)
    xpad[:, :, 1:41, 1:41] = x.astype(bf)
    # [n, cb, c, i, ki, j, kj] -> [cb, c, ki, kj, n, i, j]
    xv = xpad.reshape(B, 4, 128, 14, 3, 14, 3).transpose(1, 2, 4, 6, 0, 3, 5)

    w1 = np.asarray(inputs["conv1_w"], dtype=f)          # [256, 512, 3, 3]
    w1p = np.ascontiguousarray(
        w1.reshape(256, 4, 128, 9).transpose(2, 1, 3, 0)).reshape(128, 36, 256).astype(bf)
    w2 = np.asarray(inputs["conv2_w"], dtype=f)          # [128, 256, 3, 3]
    w2p = np.ascontiguousarray(
        w2.reshape(128, 2, 128, 9).transpose(2, 1, 3, 0)).reshape(128, 18, 128).astype(bf)

    # compose the 12 affine layers (no nonlinearities) into [3200] + scalar
    M = np.asarray(inputs["w14"], dtype=np.float64)      # [1, 2]
    beff = np.asarray(inputs["b14"], dtype=np.float64).copy()  # [1]
    for li in range(13, 2, -1):                          # w13 .. w3
        beff += M @ np.asarray(inputs[f"b{li}"], dtype=np.float64)
        M = M @ np.asarray(inputs[f"w{li}"], dtype=np.float64)
    weff = M.reshape(3200).astype(f)                     # order f = c*25 + ij
    w2d = weff.reshape(128, 25)
    weffp = np.zeros((128, 26), dtype=f)
    weffp[:, 0:25] = w2d
    weffp[:, 25] = w2d.sum(axis=1)
    beff_f = float(beff[0])

    bn1_g = np.asarray(inputs["bn1_g"], dtype=f)
    bn1_b = np.asarray(inputs["bn1_b"], dtype=f)
    bn2_g = np.asarray(inputs["bn2_g"], dtype=f)
    bn2_b = np.asarray(inputs["bn2_b"], dtype=f)

    bp = np.zeros((128, 7), dtype=f)
    bp[:, 0:2] = bn1_g.reshape(2, 128).T
    bp[:, 2:4] = bn1_b.reshape(2, 128).T
    bp[:, 4] = bn2_g
    bp[:, 5] = bn2_b
    bp[0, 6] = beff_f

    in_maps = []
    for r in range(NCORES):
        xr = np.ascontiguousarray(
            xv[:, :, :, :, r * BL:(r + 1) * BL]        # [4,128,3,3,8,14,14]
            .reshape(4, 128, 9, NPT, PTW)
            .transpose(0, 3, 1, 2, 4)                  # [4cb, 4pt, 128, 9, 392]
        ).reshape(4, NPT, 128, 9 * PTW)
        in_maps.append({
            "xprep": xr, "w1p": w1p, "w2p": w2p,
            "weffp": weffp, "bprep": bp,
        })
    return in_maps


def kernel(**inputs):
    if "nc" not in _CACHE:
        _CACHE["nc"] = _build()
    nc = _CACHE["nc"]
    in_maps = _prep_inputs(inputs)
    trace = bool(int(os.environ.get("KERNEL_TRACE", "0")))
    if trace:
        import ntff_shim
        ntff_shim.install()
    res = run_bass_kernel_spmd(nc, in_maps, core_ids=list(range(NCORES)),
                               trace=trace)
    _CACHE["last_result"] = res
    return np.concatenate([res.results[r]["out"] for r in range(NCORES)], axis=0)
